# revision 27
# baseline (speedup 1.0000x reference)
"""Multi-Head Latent Attention (MLA) Bass kernel for 8 Trainium2 NeuronCores.

Sharding:
  - latent projections (d_kv, d_q): sequence-sharded (BS/8 rows per core),
    per-latent AllGathers (overlapped with compute) replicate the latents.
  - up-projections (u_k, u_v, u_q, qr) + attention: head-sharded, 2 heads/core.
  - context: two AllToAlls (per local head, first one overlapped with the
    second head's attention) re-shard to sequence; out_proj sequence-parallel.
Precision: float32r matmuls (full-rate, ~tf32 accuracy), fp16 as collective
transfer format (engine-cast back to f32r), bf16 for probs@V / denominators,
fp32 PSUM accumulation, fp32 output.
Attention inner loop computes scoresT [sk, sq] then ctxT [dh, sq] directly
(lhsT = V tile), with softmax denominators from a ones-vector matmul and
normalization applied via a PE column-broadcast + one DVE multiply.
"""
import sys
import os

for _p in ("/opt/trn_rl_repo", "/root/.axon_site/_ro/trn_rl_repo"):
    if os.path.isdir(_p) and _p not in sys.path:
        sys.path.insert(0, _p)

import math
import numpy as np
import ml_dtypes

BF = ml_dtypes.bfloat16

import concourse.bacc as bacc
import concourse.mybir as mybir
from concourse import tile
from concourse.bass_utils import run_bass_kernel_spmd
from concourse.masks import make_identity

# problem dims (hardcoded)
B, S, H, Dh, Dr, HID, C = 2, 2048, 16, 128, 64, 2048, 512
BS = B * S                      # 4096
NCORES = 8
H_LOC = H // NCORES             # 2
S_LOC = BS // NCORES            # 512
SCALE = 1.0 / math.sqrt(Dh + Dr)

F32 = mybir.dt.float32
F32R = mybir.dt.float32r
F16 = mybir.dt.float16
BF16 = mybir.dt.bfloat16

_CACHE = {}


def _build_program():
    nc = bacc.Bacc("TRN2", target_bir_lowering=False, debug=False,
                   num_devices=NCORES)

    # ---- external inputs (f32r matmul operands; host passes fp32 arrays) ----
    xT_in = nc.dram_tensor("xT_loc", [HID, S_LOC], BF16, kind="ExternalInput")
    wdkvT = nc.dram_tensor("wdkvT", [HID, C], BF16, kind="ExternalInput")
    wdqT = nc.dram_tensor("wdqT", [HID, C], BF16, kind="ExternalInput")
    bdkv = nc.dram_tensor("bdkv", [4, 128, 1], F32, kind="ExternalInput")
    bdq = nc.dram_tensor("bdq", [4, 128, 1], F32, kind="ExternalInput")
    ukT = nc.dram_tensor("ukT", [C, 256], BF16, kind="ExternalInput")
    uqT = nc.dram_tensor("uqT", [C, 256], BF16, kind="ExternalInput")
    uvT = nc.dram_tensor("uvT", [C, 256], BF16, kind="ExternalInput")
    buk = nc.dram_tensor("buk", [2, 128, 1], F32, kind="ExternalInput")
    buq = nc.dram_tensor("buq", [2, 128, 1], F32, kind="ExternalInput")
    buv = nc.dram_tensor("buv", [1, 256], BF16, kind="ExternalInput")
    wrAq = nc.dram_tensor("wrAq", [C, 128], BF16, kind="ExternalInput")
    wrBq = nc.dram_tensor("wrBq", [C, 128], BF16, kind="ExternalInput")
    wrAk = nc.dram_tensor("wrAk", [C, 128], BF16, kind="ExternalInput")
    wrBk = nc.dram_tensor("wrBk", [C, 128], BF16, kind="ExternalInput")
    bAq = nc.dram_tensor("bAq", [1, 128], BF16, kind="ExternalInput")
    bBq = nc.dram_tensor("bBq", [1, 128], BF16, kind="ExternalInput")
    bAk = nc.dram_tensor("bAk", [1, 128], BF16, kind="ExternalInput")
    bBk = nc.dram_tensor("bBk", [1, 128], BF16, kind="ExternalInput")
    c1_in = nc.dram_tensor("c1", [128, S], F16, kind="ExternalInput")
    c2_in = nc.dram_tensor("c2", [128, S], F16, kind="ExternalInput")
    
    woT = nc.dram_tensor("woT", [H * Dh, HID], BF16, kind="ExternalInput")
    out_sl = nc.dram_tensor("out_slice", [S_LOC, HID], F32, kind="ExternalOutput")

    groups = [list(range(NCORES))]
    EXP = mybir.ActivationFunctionType.Exp
    IDENT = mybir.ActivationFunctionType.Identity

    with tile.TileContext(nc) as tc:
        with tc.tile_pool(name="dram", bufs=1, space="DRAM") as dram:
            kv_ag_i = dram.tile([4, 128, S_LOC], BF16)
            kv_ag_o = dram.tile([NCORES, 4, 128, S_LOC], BF16,
                                name="kvago")
            ql_ag_i = dram.tile([4, 128, S_LOC], BF16)
            ql_ag_o = dram.tile([NCORES, 4, 128, S_LOC], BF16,
                                name="qlago")
            a2a_i = dram.tile([NCORES, H_LOC * Dh, S_LOC], BF16)
            a2a_o = dram.tile([NCORES, H_LOC * Dh, S_LOC], BF16)

            with tc.tile_pool(name="const", bufs=1) as const:
                ones_row = const.tile([1, 512], BF16)
                nc.vector.memset(ones_row[:], 1.0)
                ones_col = const.tile([128, 1], BF16)
                nc.vector.memset(ones_col[:], 1.0)
                ident = const.tile([128, 128], F16)
                make_identity(nc, ident[:])

                # ============ Phase A: latents (sequence-sharded) ============
                with tc.tile_pool(name="phA", bufs=1) as phA, \
                     tc.tile_pool(name="phAb", bufs=4) as phAb, \
                     tc.tile_pool(name="psA", bufs=4, space="PSUM") as psA:
                    xts, wkv, wql = [], [], []
                    for ht in range(16):
                        xt = phA.tile([128, S_LOC], BF16, tag=f"xt{ht}",
                                      name=f"xt{ht}")
                        nc.sync.dma_start(out=xt[:],
                                          in_=xT_in[ht * 128:(ht + 1) * 128, :])
                        xts.append(xt)
                        w = phA.tile([128, C], BF16, tag=f"wk{ht}",
                                     name=f"wk{ht}")
                        nc.sync.dma_start(out=w[:],
                                          in_=wdkvT[ht * 128:(ht + 1) * 128, :])
                        wkv.append(w)
                    for ht in range(16):
                        w = phA.tile([128, C], BF16, tag=f"wq{ht}",
                                     name=f"wq{ht}")
                        nc.sync.dma_start(out=w[:],
                                          in_=wdqT[ht * 128:(ht + 1) * 128, :])
                        wql.append(w)
                    for li, (wfull, bdram, agi, ago) in enumerate(
                            ((wkv, bdkv, kv_ag_i, kv_ag_o),
                             (wql, bdq, ql_ag_i, ql_ag_o))):
                        for ct in range(4):
                            ps = psA.tile([128, S_LOC], F32, tag="ps", name="psa")
                            for ht in range(16):
                                nc.tensor.matmul(
                                    ps[:],
                                    wfull[ht][:, ct * 128:(ct + 1) * 128],
                                    xts[ht][:],
                                    start=(ht == 0), stop=(ht == 15))
                            bt = phAb.tile([128, 1], F32, tag="blat", name="blat")
                            nc.sync.dma_start(out=bt[:], in_=bdram[ct])
                            lat = phAb.tile([128, S_LOC], BF16, tag="lat",
                                            bufs=3, name="lat")
                            nc.vector.tensor_scalar_add(lat[:], ps[:], bt[:])
                            nc.sync.dma_start(out=agi[ct], in_=lat[:])
                        nc.gpsimd.collective_compute(
                            "AllGather", mybir.AluOpType.bypass,
                            replica_groups=groups,
                            ins=[agi.opt()], outs=[ago.opt()])

                # ====== persistent attention operand tiles (phases B-C) ======
                with tc.tile_pool(name="attn", bufs=1) as attn:
                    kcT = [attn.tile([128, BS], BF16, tag=f"kcT{h}",
                                     name=f"kcT{h}") for h in range(2)]
                    qcT = [attn.tile([128, BS], BF16, tag=f"qcT{h}",
                                     name=f"qcT{h}") for h in range(2)]
                    krT = attn.tile([128, BS], BF16, tag="krT", name="krT")
                    qrT = attn.tile([128, BS], BF16, tag="qrT", name="qrT")
                    v_sb = attn.tile([128, 32, 258], BF16, tag="v", name="v_sb")

                    # ============ Phase B: up-projections + RoPE ============
                    with tc.tile_pool(name="phB", bufs=1) as phB, \
                         tc.tile_pool(name="phBt", bufs=3) as phBt, \
                         tc.tile_pool(name="psB", bufs=2, space="PSUM") as psB:
                        c1s = phB.tile([128, S], F16, tag="c1", name="c1s")
                        nc.sync.dma_start(out=c1s[:], in_=c1_in[:])
                        c2s = phB.tile([128, S], F16, tag="c2", name="c2s")
                        nc.sync.dma_start(out=c2s[:], in_=c2_in[:])
                        buv_s = phB.tile([1, 256], BF16, tag="buv", name="buv_s")
                        nc.sync.dma_start(out=buv_s[:], in_=buv[:])
                        bias_r = {}
                        for nm, t in (("bAq", bAq), ("bBq", bBq),
                                      ("bAk", bAk), ("bBk", bBk)):
                            bs_ = phB.tile([1, 128], BF16, tag=nm, name=nm + "s")
                            nc.sync.dma_start(out=bs_[:], in_=t[:])
                            bias_r[nm] = bs_
                        upw = {}
                        for nm, t, w_ in (("uk", ukT, 256), ("uq", uqT, 256),
                                          ("uv", uvT, 256), ("wrAq", wrAq, 128),
                                          ("wrBq", wrBq, 128), ("wrAk", wrAk, 128),
                                          ("wrBk", wrBk, 128)):
                            tl = []
                            for ct in range(4):
                                wt = phB.tile([128, w_], BF16, tag=f"{nm}{ct}",
                                              name=f"{nm}{ct}")
                                nc.sync.dma_start(
                                    out=wt[:],
                                    in_=t[ct * 128:(ct + 1) * 128, :])
                                tl.append(wt)
                            upw[nm] = tl
                        bukq = {}
                        for nm, t in (("buk", buk), ("buq", buq)):
                            tl = []
                            for h in range(2):
                                bt_ = phB.tile([128, 1], F32, tag=f"{nm}{h}",
                                               name=f"{nm}{h}")
                                nc.sync.dma_start(out=bt_[:], in_=t[h])
                                tl.append(bt_)
                            bukq[nm] = tl

                        def rproj(ps_tag, wa, ba, src):
                            ps_ = psB.tile([128, 512], F32, tag=ps_tag,
                                           name=ps_tag)
                            nc.tensor.matmul(ps_[:], bias_r[ba][:],
                                             ones_row[:, :512],
                                             start=True, stop=False)
                            for ct in range(4):
                                nc.tensor.matmul(ps_[:], upw[wa][ct][:],
                                                 src[ct][:],
                                                 start=False, stop=(ct == 3))
                            return ps_

                        def load_lat(ago, tagp):
                            tiles = []
                            for ct in range(4):
                                f16t = phBt.tile([128, 512], F16,
                                                 tag=f"{tagp}h{ct}", bufs=2,
                                                 name=f"{tagp}h{ct}")
                                nc.sync.dma_start(out=f16t[:], in_=ago)
                                tiles.append(f16t)
                            return tiles

                        # ---- pass 1: kv-dependent (k_c, rope-k, V) ----
                        for j2 in range(8):
                            sl = slice(j2 * 512, (j2 + 1) * 512)
                            pos = slice((j2 % 4) * 512, (j2 % 4) * 512 + 512)
                            kv_sb = []
                            for ct in range(4):
                                kt_ = phBt.tile([128, 512], BF16,
                                                tag=f"kv{ct}", bufs=2,
                                                name=f"kv{ct}")
                                nc.sync.dma_start(out=kt_[:],
                                                  in_=kv_ag_o[j2, ct])
                                kv_sb.append(kt_)
                            for h in range(2):
                                hc = slice(h * 128, (h + 1) * 128)
                                ps = psB.tile([128, 512], F32, tag="psKC",
                                              name="pskc")
                                for ct in range(4):
                                    nc.tensor.matmul(ps[:],
                                                     upw["uk"][ct][:, hc],
                                                     kv_sb[ct][:],
                                                     start=(ct == 0),
                                                     stop=(ct == 3))
                                nc.scalar.activation(
                                    kcT[h][:, sl], ps[:], IDENT,
                                    bias=bukq["buk"][h][:])
                            psa_ = rproj("psRA", "wrAk", "bAk", kv_sb)
                            psb_ = rproj("psRB", "wrBk", "bBk", kv_sb)
                            t1 = phBt.tile([128, 512], F32, tag="t1", bufs=2, name="t1")
                            nc.vector.tensor_mul(t1[:], psa_[:], c1s[:, pos])
                            t2 = phBt.tile([128, 512], F32, tag="t2", bufs=2, name="t2")
                            nc.vector.tensor_mul(t2[:], psb_[:], c2s[:, pos])
                            nc.vector.tensor_add(krT[:, sl], t1[:], t2[:])
                            for ss in range(4):
                                psv = psB.tile([128, 256], F32, tag="psV",
                                               name="psv")
                                nc.tensor.matmul(psv[:], ones_row[:, :128],
                                                 buv_s[:], start=True,
                                                 stop=False)
                                ssl = slice(ss * 128, (ss + 1) * 128)
                                for ct in range(4):
                                    nc.tensor.matmul(psv[:],
                                                     kv_sb[ct][:, ssl],
                                                     upw["uv"][ct][:],
                                                     start=False, stop=(ct == 3))
                                st = j2 * 4 + ss
                                nc.scalar.copy(v_sb[:, st, 0:128], psv[:, 0:128])
                                nc.scalar.copy(v_sb[:, st, 129:257],
                                               psv[:, 128:256])
                                nc.vector.memset(v_sb[:, st, 128:129], 1.0)
                                nc.vector.memset(v_sb[:, st, 257:258], 1.0)

                        # ---- pass 2: ql-dependent (q_c, rope-q) ----
                        for j2 in range(8):
                            sl = slice(j2 * 512, (j2 + 1) * 512)
                            pos = slice((j2 % 4) * 512, (j2 % 4) * 512 + 512)
                            ql_sb = []
                            for ct in range(4):
                                qt_ = phBt.tile([128, 512], BF16,
                                                tag=f"ql{ct}", bufs=2,
                                                name=f"ql{ct}")
                                nc.sync.dma_start(out=qt_[:],
                                                  in_=ql_ag_o[j2, ct])
                                ql_sb.append(qt_)
                            for h in range(2):
                                hc = slice(h * 128, (h + 1) * 128)
                                ps = psB.tile([128, 512], F32, tag="psKC",
                                              name="psqc")
                                for ct in range(4):
                                    nc.tensor.matmul(ps[:],
                                                     upw["uq"][ct][:, hc],
                                                     ql_sb[ct][:],
                                                     start=(ct == 0),
                                                     stop=(ct == 3))
                                nc.scalar.activation(
                                    qcT[h][:, sl], ps[:], IDENT,
                                    bias=bukq["buq"][h][:])
                            psa_ = rproj("psRA", "wrAq", "bAq", ql_sb)
                            psb_ = rproj("psRB", "wrBq", "bBq", ql_sb)
                            t1 = phBt.tile([128, 512], F32, tag="t1", bufs=2, name="t1")
                            nc.vector.tensor_mul(t1[:], psa_[:], c1s[:, pos])
                            t2 = phBt.tile([128, 512], F32, tag="t2", bufs=2, name="t2")
                            nc.vector.tensor_mul(t2[:], psb_[:], c2s[:, pos])
                            nc.vector.tensor_add(qrT[:, sl], t1[:], t2[:])

                    # ================= Phase C: attention =================
                    # per (b, sqb) block covering BOTH local heads: the two
                    # K=64 rotary matmuls land on disjoint PE row groups and
                    # run concurrently. Normalize+transpose pipelined one
                    # block behind the scores to keep the PE stream dense.
                    with tc.tile_pool(name="phC", bufs=1) as phC, \
                         tc.tile_pool(name="psC", bufs=1, space="PSUM") as psC:
                        def flush_ctx(pend):
                            pb, psqb, ctxns = pend
                            g = pb * 4 + psqb
                            for h in range(2):
                                stg = phC.tile([128, 512], BF16, tag="stg",
                                               bufs=2, name="stg")
                                for sqs in range(4):
                                    pst = psC.tile([128, 128], F16, tag="psX",
                                                   bufs=2, name="pst")
                                    nc.tensor.transpose(pst[:],
                                                        ctxns[h][sqs][:],
                                                        ident[:])
                                    nc.vector.tensor_copy(
                                        stg[:, sqs * 128:(sqs + 1) * 128],
                                        pst[:])
                                nc.sync.dma_start(
                                    out=a2a_i[g, h * 128:(h + 1) * 128, :],
                                    in_=stg[:])

                        pending = None
                        for b in range(2):
                            for sqb in range(4):
                                qsl = slice(b * S + sqb * 512,
                                            b * S + sqb * 512 + 512)
                                probs2 = [phC.tile([128, 16, 512], BF16,
                                                   tag=f"probs{h}", bufs=2,
                                                   name=f"probs{h}")
                                          for h in range(2)]
                                for st2 in range(8):
                                    ps2 = [psC.tile([128, 2, 512], F32,
                                                    tag="psS2", bufs=3,
                                                    name=f"ps2{h}")
                                           for h in range(2)]
                                    for p in range(2):
                                        skt = st2 * 2 + p
                                        ksl = slice(b * S + skt * 128,
                                                    b * S + skt * 128 + 128)
                                        for h in range(2):
                                            nc.tensor.matmul(
                                                ps2[h][:, p, :],
                                                kcT[h][:, ksl],
                                                qcT[h][:, qsl],
                                                start=True, stop=False)
                                        # K=64 rotary: disjoint row groups ->
                                        # the two heads run concurrently
                                        for h in range(2):
                                            hr = slice(h * 64, h * 64 + 64)
                                            nc.tensor.matmul(
                                                ps2[h][:, p, :],
                                                krT[hr, ksl],
                                                qrT[hr, qsl],
                                                start=False, stop=True)
                                    for h in range(2):
                                        nc.scalar.activation(
                                            probs2[h][:, st2 * 2:st2 * 2 + 2,
                                                      :],
                                            ps2[h][:], EXP)
                                ctxns = [[], []]
                                for h in range(2):
                                    for sqs in range(4):
                                        psx = psC.tile([128, 132], F32,
                                                       tag="psX", bufs=2,
                                                       name="psx")
                                        for skt in range(16):
                                            vt = b * 16 + skt
                                            nc.tensor.matmul(
                                                psx[:, 0:129],
                                                probs2[h][:, skt,
                                                          sqs * 128:
                                                          (sqs + 1) * 128],
                                                v_sb[:, vt,
                                                     h * 129:h * 129 + 129],
                                                start=(skt == 0),
                                                stop=(skt == 15))
                                        rec = phC.tile([128, 1], F32,
                                                       tag="rec", bufs=4,
                                                       name="rec")
                                        nc.vector.reciprocal(
                                            rec[:], psx[:, 128:129])
                                        ctxn = phC.tile([128, 128], F16,
                                                        tag="ctxn", bufs=16,
                                                        name="ctxn")
                                        nc.vector.tensor_scalar_mul(
                                            ctxn[:], psx[:, 0:128], rec[:])
                                        ctxns[h].append(ctxn)
                                if pending is not None:
                                    flush_ctx(pending)
                                pending = (b, sqb, ctxns)
                        flush_ctx(pending)
                        nc.gpsimd.collective_compute(
                            "AllToAll", mybir.AluOpType.bypass,
                            replica_groups=groups,
                            ins=[a2a_i.opt()], outs=[a2a_o.opt()])

                # ================= Phase D: out projection =================
                with tc.tile_pool(name="phD", bufs=1) as phD, \
                     tc.tile_pool(name="phDw", bufs=2) as phDw, \
                     tc.tile_pool(name="phDo", bufs=3) as phDo, \
                     tc.tile_pool(name="psD", bufs=2, space="PSUM") as psD:
                    csl = []
                    for dht in range(16):
                        cf = phD.tile([128, S_LOC], BF16, tag=f"cf{dht}",
                                      name=f"cf{dht}")
                        nc.sync.dma_start(
                            out=cf[:],
                            in_=a2a_o[dht // 2,
                                      (dht % 2) * 128:(dht % 2) * 128 + 128, :])
                        csl.append(cf)
                    for ot in range(4):
                        osl = slice(ot * 512, (ot + 1) * 512)
                        wos = []
                        for dht in range(16):
                            wo = phDw.tile([128, 512], BF16, tag=f"wo{dht}",
                                           name=f"wo{dht}")
                            nc.sync.dma_start(
                                out=wo[:],
                                in_=woT[dht * 128:(dht + 1) * 128, osl])
                            wos.append(wo)
                        for ssub in range(4):
                            pso = psD.tile([128, 512], F32, tag="psO",
                                           name="pso")
                            ssl = slice(ssub * 128, (ssub + 1) * 128)
                            for dht in range(16):
                                nc.tensor.matmul(pso[:], csl[dht][:, ssl],
                                                 wos[dht][:],
                                                 start=(dht == 0),
                                                 stop=(dht == 15))
                            osb = phDo.tile([128, 512], F32, tag="osb",
                                            name="osb")
                            nc.vector.tensor_copy(osb[:], pso[:])
                            nc.sync.dma_start(out=out_sl[ssl, osl], in_=osb[:])

    nc.compile()
    return nc


def _host_prep(inputs):
    """Build per-core input maps from the full problem inputs."""
    x = np.asarray(inputs["x"], np.float32)
    xT = np.ascontiguousarray(x.reshape(BS, HID).T)            # [HID, BS]
    wdkvT = np.ascontiguousarray(np.asarray(inputs["d_kv_w"], np.float32).T.astype(BF))
    wdqT = np.ascontiguousarray(np.asarray(inputs["d_q_w"], np.float32).T.astype(BF))
    bdkv = np.asarray(inputs["d_kv_b"], np.float32).reshape(4, 128, 1)
    bdq = np.asarray(inputs["d_q_b"], np.float32).reshape(4, 128, 1)

    uk3 = np.asarray(inputs["u_k_w"], np.float32).reshape(H, Dh, C)
    uq3 = np.asarray(inputs["u_q_w"], np.float32).reshape(H, Dh, C) * SCALE
    uv3 = np.asarray(inputs["u_v_w"], np.float32).reshape(H, Dh, C)
    buk2 = np.asarray(inputs["u_k_b"], np.float32).reshape(H, Dh)
    buq2 = np.asarray(inputs["u_q_b"], np.float32).reshape(H, Dh) * SCALE
    buv2 = np.asarray(inputs["u_v_b"], np.float32).reshape(H, Dh)
    qr3 = np.asarray(inputs["qr_w"], np.float32).reshape(H, Dr, C)
    bqr2 = np.asarray(inputs["qr_b"], np.float32).reshape(H, Dr)

    # rope tables (positions 0..S-1)
    i32 = np.arange(32, dtype=np.float32)
    inv_freq = (10000.0 ** (-(2.0 * i32) / Dr)).astype(np.float32)  # [32]
    pos = np.arange(S, dtype=np.float32)
    ang = pos[None, :] * inv_freq[:, None]                     # [32, S]
    cos, sin = np.cos(ang), np.sin(ang)
    c1 = np.concatenate([cos, sin, cos, sin], 0).astype(np.float32)
    c2 = np.concatenate([-sin, cos, -sin, cos], 0).astype(np.float32)

    woT = np.ascontiguousarray(np.asarray(inputs["out_w"], np.float32).T.astype(BF))

    in_maps = []
    for j in range(NCORES):
        hs = [2 * j, 2 * j + 1]
        ukT_l = uk3[hs].transpose(2, 0, 1).reshape(C, 256)
        uqT_l = uq3[hs].transpose(2, 0, 1).reshape(C, 256)
        uvT_l = uv3[hs].transpose(2, 0, 1).reshape(C, 256)
        we = [qr3[h, 0::2, :] for h in hs]    # [32, C] each
        wo = [qr3[h, 1::2, :] for h in hs]
        wrA = np.concatenate([we[0], we[0], we[1], we[1]], 0).T  # [C, 128]
        wrB = np.concatenate([wo[0], wo[0], wo[1], wo[1]], 0).T
        be = [bqr2[h, 0::2] for h in hs]
        bo = [bqr2[h, 1::2] for h in hs]
        bA = np.concatenate([be[0], be[0], be[1], be[1]])[None, :]  # [1,128]
        bB = np.concatenate([bo[0], bo[0], bo[1], bo[1]])[None, :]
        in_maps.append({
            "xT_loc": np.ascontiguousarray(
                xT[:, j * S_LOC:(j + 1) * S_LOC]).astype(BF),
            "wdkvT": wdkvT, "wdqT": wdqT, "bdkv": bdkv, "bdq": bdq,
            "ukT": np.ascontiguousarray(ukT_l.astype(BF)),
            "uqT": np.ascontiguousarray(uqT_l.astype(BF)),
            "uvT": np.ascontiguousarray(uvT_l.astype(BF)),
            "buk": buk2[hs].reshape(2, 128, 1).copy(),
            "buq": buq2[hs].reshape(2, 128, 1).copy(),
            "buv": buv2[hs].reshape(1, 256).astype(BF),
            "wrAq": np.ascontiguousarray((wrA * SCALE).astype(BF)),
            "wrBq": np.ascontiguousarray((wrB * SCALE).astype(BF)),
            "wrAk": np.ascontiguousarray(wrA.astype(BF)),
            "wrBk": np.ascontiguousarray(wrB.astype(BF)),
            "bAq": np.ascontiguousarray((bA * SCALE).astype(BF)),
            "bBq": np.ascontiguousarray((bB * SCALE).astype(BF)),
            "bAk": np.ascontiguousarray(bA.astype(BF)),
            "bBk": np.ascontiguousarray(bB.astype(BF)),
            "c1": c1.astype(np.float16), "c2": c2.astype(np.float16),
            "woT": woT,
        })
    return in_maps


def kernel(**inputs):
    if "nc" not in _CACHE:
        _CACHE["nc"] = _build_program()
    nc = _CACHE["nc"]
    in_maps = _host_prep(inputs)
    res = run_bass_kernel_spmd(nc, in_maps, list(range(NCORES)))
    out = np.concatenate([res.results[j]["out_slice"] for j in range(NCORES)], 0)
    out = out + np.asarray(inputs["out_b"], np.float32)[None, :]
    return out.reshape(B, S, HID)
